# revision 2
# baseline (speedup 1.0000x reference)
"""Trainium2 Bass kernel for nn_Encoder_Flows (3-layer dense GCN message passing).

Math per graph (reference):
    A = flows [N, N];  deg[c] = sum_r A[r, c];  dinv = rsqrt(deg)
    L(x, W, b) = dinv * (A^T @ (dinv * (x @ W))) + b
    out = L(L(L(A, W1, b1), W2, b2), W3, b3)          # [N, 128]

Strategy: data-parallel over the batch (16 graphs / 8 cores = 2 graphs per
core, processed sequentially inside one NEFF). A is cast to bf16 on the host
and kept resident in SBUF (double-buffered across graphs). The layer-1
feature matmul A @ W1 needs A-transposed tiles; those come from hardware
DMA-transpose (bf16-only feature) streamed per 128-column strip. All other
stages pick matmul operand roles so that no on-chip transpose is ever
needed:
  u1   = A @ W1          : lhsT = At-strip tiles (stationary), rhs = W1  -> node-major
  y1   = A^T @ msg1      : lhsT = msg1, rhs = A (N=512 chunks)           -> feat-major
  v2   = y1 @ W2         : lhsT = y1T tiles (feat-major is the lhsT!)    -> node-major
  y2   = A^T @ msg2      : like y1, two 128-col halves                   -> feat-major
  v3   = y2 @ W3         : lhsT = y2T tiles                              -> node-major
  y3   = A^T @ msg3      : lhsT = A tiles (stationary), rhs = msg3       -> node-major
deg comes free as a DVE free-axis reduction over the DMA-transposed strips.
All matmuls accumulate in fp32 PSUM; dinv/scalings in fp32.
"""

import sys
from contextlib import ExitStack

import numpy as np

for _p in ("/opt/trn_rl_repo", "/opt/pypackages"):
    if _p not in sys.path:
        sys.path.append(_p)

import ml_dtypes

B, N, P = 16, 2048, 128
NB = N // P          # 16 row/col blocks
NCORES = 8
GPC = B // NCORES    # graphs per core
D1, D2, D3 = 128, 256, 128
CH = 512             # moving-operand chunk
NCH = N // CH

_COMPILED = {}


def _build(with_bias):
    import concourse.mybir as mybir
    import concourse.tile as tile
    from concourse import bacc

    f32 = mybir.dt.float32
    bf16 = mybir.dt.bfloat16

    nc = bacc.Bacc("TRN2", target_bir_lowering=False)
    Ab_d = nc.declare_dram_parameter("Ab", [GPC, N, N], bf16, isOutput=False)
    W1_d = nc.declare_dram_parameter("W1b", [N, D1], bf16, isOutput=False)
    W2_d = nc.declare_dram_parameter("W2b", [D1, D2], bf16, isOutput=False)
    W3_d = nc.declare_dram_parameter("W3b", [D2, D3], bf16, isOutput=False)
    if with_bias:
        c2_d = nc.declare_dram_parameter("c2r", [P, D2], f32, isOutput=False)
        c3_d = nc.declare_dram_parameter("c3r", [P, D3], f32, isOutput=False)
        b3_d = nc.declare_dram_parameter("b3r", [P, D3], f32, isOutput=False)
    out_d = nc.declare_dram_parameter("out", [GPC, N, D3], f32, isOutput=True)

    with tile.TileContext(nc) as tc, ExitStack() as ctx:
        X = mybir.AxisListType.X
        wpool = ctx.enter_context(tc.tile_pool(name="wpool", bufs=1))
        spool = ctx.enter_context(tc.tile_pool(name="spool", bufs=2))
        apool = ctx.enter_context(tc.tile_pool(name="apool", bufs=2))
        strips = ctx.enter_context(tc.tile_pool(name="strips", bufs=3))
        mpool = ctx.enter_context(tc.tile_pool(name="mpool", bufs=1))
        y2pool = ctx.enter_context(tc.tile_pool(name="y2pool", bufs=1))
        psum = ctx.enter_context(tc.tile_pool(name="psum", bufs=1, space="PSUM"))
        psmall = ctx.enter_context(tc.tile_pool(name="psmall", bufs=2, space="PSUM"))

        # --- weights, replicated constants ---
        W1_sb = wpool.tile([P, NB, D1], bf16)
        nc.sync.dma_start(W1_sb[:], W1_d.ap().rearrange("(fb p) d -> p fb d", p=P))
        W2_sb = wpool.tile([P, D2], bf16)
        nc.sync.dma_start(W2_sb[:], W2_d.ap())
        W3_sb = wpool.tile([P, 2, D3], bf16)
        nc.sync.dma_start(W3_sb[:], W3_d.ap().rearrange("(h p) g -> p h g", p=P))
        if with_bias:
            c2_sb = wpool.tile([P, D2], f32)
            nc.sync.dma_start(c2_sb[:], c2_d.ap())
            c3_sb = wpool.tile([P, D3], f32)
            nc.sync.dma_start(c3_sb[:], c3_d.ap())
            b3_sb = wpool.tile([P, D3], f32)
            nc.sync.dma_start(b3_sb[:], b3_d.ap())

        iob = wpool.tile([P, P], bf16)
        from concourse.masks import make_identity
        make_identity(nc, iob[:])

        out_ap = out_d.ap().rearrange("g (cb p) d -> g p cb d", p=P)

        for g in range(GPC):
            # ---------- load A (bf16, natural layout) ----------
            A_sb = apool.tile([P, NB, N], bf16, tag="A")
            nc.sync.dma_start(A_sb[:], Ab_d.ap()[g].rearrange("(rb p) c -> p rb c", p=P))

            deg = spool.tile([P, NB], f32, tag="deg")
            dinv = spool.tile([P, NB], f32, tag="dinv")
            rdeg = spool.tile([P, NB], f32, tag="rdeg")

            # ---------- u1T = (A @ W1)^T via DMA-transposed strips; deg free --
            # u1T[d, m] accumulates over fb: lhsT = W1[fb], rhs = At-strip chunks
            u1t = psum.tile([P, N], f32, tag="big")
            for fb in range(NB):
                strip = strips.tile([P, N], bf16, tag="strip")
                nc.sync.dma_start_transpose(strip[:], Ab_d.ap()[g][:, fb * P:(fb + 1) * P])
                nc.vector.reduce_sum(deg[:, fb:fb + 1], strip[:], axis=X)
                for ch in range(NCH):
                    nc.tensor.matmul(
                        u1t[:, ch * CH:(ch + 1) * CH], W1_sb[:, fb, :],
                        strip[:, ch * CH:(ch + 1) * CH],
                        start=(fb == 0), stop=(fb == NB - 1))

            # dinv = sqrt(1/deg); rdeg = 1/deg = dinv^2
            nc.vector.reciprocal(rdeg[:], deg[:])
            nc.scalar.sqrt(dinv[:], rdeg[:])

            # ---------- msg1 = dinv * u1 (node-major via 16 PE transposes) ----
            msg1 = mpool.tile([P, NB, D1], bf16, tag="msg")
            for q in range(4):
                u1q = spool.tile([P, CH], bf16, tag="u1q")
                nc.vector.tensor_copy(u1q[:], u1t[:, q * CH:(q + 1) * CH])
                pt = psum.tile([P, 4, P], bf16, tag="quad")
                for j in range(4):
                    nc.tensor.transpose(pt[:, j, :], u1q[:, j * P:(j + 1) * P], iob[:])
                sl = slice(q * 4, (q + 1) * 4)
                nc.vector.tensor_tensor(
                    msg1[:, sl, :], pt[:],
                    dinv[:, sl, None].to_broadcast([P, 4, D1]),
                    mybir.AluOpType.mult)

            # ---------- y1 = A^T @ msg1  (feat-major out) ----------
            y1 = psum.tile([P, N], f32, tag="big")
            for ch in range(NCH):
                for rb in range(NB):
                    nc.tensor.matmul(
                        y1[:, ch * CH:(ch + 1) * CH], msg1[:, rb, :],
                        A_sb[:, rb, ch * CH:(ch + 1) * CH],
                        start=(rb == 0), stop=(rb == NB - 1))

            # ---------- v2 = y1 @ W2 ; msg2 = rdeg*v2 (+ dinv*c2) ----------
            msg2 = mpool.tile([P, NB, D2], bf16, tag="msg2")
            for q in range(4):
                y1q = spool.tile([P, CH], bf16, tag="y1q")
                nc.vector.tensor_copy(y1q[:], y1[:, q * CH:(q + 1) * CH])
                for j in range(4):
                    nb = q * 4 + j
                    v2p = psmall.tile([P, D2], f32, tag="small")
                    nc.tensor.matmul(v2p[:], y1q[:, j * P:(j + 1) * P], W2_sb[:],
                                     start=True, stop=True)
                    if with_bias:
                        t = spool.tile([P, D2], f32, tag="tbias")
                        nc.vector.tensor_tensor(
                            t[:], v2p[:], dinv[:, nb:nb + 1].to_broadcast([P, D2]),
                            mybir.AluOpType.mult)
                        nc.vector.tensor_tensor(t[:], t[:], c2_sb[:], mybir.AluOpType.add)
                        nc.vector.tensor_tensor(
                            msg2[:, nb, :], t[:], dinv[:, nb:nb + 1].to_broadcast([P, D2]),
                            mybir.AluOpType.mult)
                    else:
                        nc.vector.tensor_tensor(
                            msg2[:, nb, :], v2p[:], rdeg[:, nb:nb + 1].to_broadcast([P, D2]),
                            mybir.AluOpType.mult)

            # ---------- y2 = A^T @ msg2 (two 128-feat halves) ----------
            y2h = []
            for half in range(2):
                y2p = psum.tile([P, N], f32, tag="big")
                for ch in range(NCH):
                    for rb in range(NB):
                        nc.tensor.matmul(
                            y2p[:, ch * CH:(ch + 1) * CH],
                            msg2[:, rb, half * P:(half + 1) * P],
                            A_sb[:, rb, ch * CH:(ch + 1) * CH],
                            start=(rb == 0), stop=(rb == NB - 1))
                yh = y2pool.tile([P, N], bf16, tag=f"y2h{half}")
                nc.vector.tensor_copy(yh[:], y2p[:])
                y2h.append(yh)

            # ---------- v3 = y2 @ W3 ; msg3 = rdeg*v3 (+ dinv*c3) ----------
            msg3 = mpool.tile([P, NB, D3], bf16, tag="msg")
            for nb in range(NB):
                v3p = psmall.tile([P, D3], f32, tag="small")
                for half in range(2):
                    nc.tensor.matmul(v3p[:], y2h[half][:, nb * P:(nb + 1) * P],
                                     W3_sb[:, half, :],
                                     start=(half == 0), stop=(half == 1))
                if with_bias:
                    t3 = spool.tile([P, D3], f32, tag="tbias3")
                    nc.vector.tensor_tensor(
                        t3[:], v3p[:], dinv[:, nb:nb + 1].to_broadcast([P, D3]),
                        mybir.AluOpType.mult)
                    nc.vector.tensor_tensor(t3[:], t3[:], c3_sb[:], mybir.AluOpType.add)
                    nc.vector.tensor_tensor(
                        msg3[:, nb, :], t3[:], dinv[:, nb:nb + 1].to_broadcast([P, D3]),
                        mybir.AluOpType.mult)
                else:
                    nc.vector.tensor_tensor(
                        msg3[:, nb, :], v3p[:], rdeg[:, nb:nb + 1].to_broadcast([P, D3]),
                        mybir.AluOpType.mult)

            # ---------- y3 = A^T @ msg3, A-stationary -> node-major ----------
            y3 = psum.tile([P, NB, D3], f32, tag="big")
            for cb in range(NB):
                for rb in range(NB):
                    nc.tensor.matmul(
                        y3[:, cb, :], A_sb[:, rb, cb * P:(cb + 1) * P], msg3[:, rb, :],
                        start=(rb == 0), stop=(rb == NB - 1))

            # ---------- out = dinv*y3 (+ b3) ----------
            for q in range(4):
                sl = slice(q * 4, (q + 1) * 4)
                og = spool.tile([P, 4, D3], f32, tag="og")
                nc.vector.tensor_tensor(
                    og[:], y3[:, sl, :],
                    dinv[:, sl, None].to_broadcast([P, 4, D3]),
                    mybir.AluOpType.mult)
                if with_bias:
                    nc.vector.tensor_tensor(
                        og[:], og[:], b3_sb[:, None, :].to_broadcast([P, 4, D3]),
                        mybir.AluOpType.add)
                nc.sync.dma_start(out_ap[g][:, sl, :], og[:])

    nc.compile()
    return nc


def _get_nc(with_bias):
    key = bool(with_bias)
    if key not in _COMPILED:
        _COMPILED[key] = _build(key)
    return _COMPILED[key]


def kernel(flows, W1, b1, W2, b2, W3, b3, _trace=False):
    from concourse.bass_utils import run_bass_kernel_spmd

    flows = np.asarray(flows, dtype=np.float32)
    W1 = np.asarray(W1, dtype=np.float32)
    W2 = np.asarray(W2, dtype=np.float32)
    W3 = np.asarray(W3, dtype=np.float32)
    b1 = np.asarray(b1, dtype=np.float32)
    b2 = np.asarray(b2, dtype=np.float32)
    b3 = np.asarray(b3, dtype=np.float32)

    with_bias = bool(np.any(b1) or np.any(b2) or np.any(b3))
    nc = _get_nc(with_bias)

    Ab = flows.astype(ml_dtypes.bfloat16)
    W1b = W1.astype(ml_dtypes.bfloat16)
    W2b = W2.astype(ml_dtypes.bfloat16)
    W3b = W3.astype(ml_dtypes.bfloat16)

    in_maps = []
    for c in range(NCORES):
        m = {
            "Ab": Ab[c * GPC:(c + 1) * GPC],
            "W1b": W1b, "W2b": W2b, "W3b": W3b,
        }
        if with_bias:
            m["c2r"] = np.broadcast_to(b1 @ W2, (P, D2)).copy().astype(np.float32)
            m["c3r"] = np.broadcast_to(b2 @ W3, (P, D3)).copy().astype(np.float32)
            m["b3r"] = np.broadcast_to(b3, (P, D3)).copy().astype(np.float32)
        in_maps.append(m)

    res = run_bass_kernel_spmd(nc, in_maps, core_ids=list(range(NCORES)), trace=_trace)
    out = np.concatenate([res.results[c]["out"] for c in range(NCORES)], axis=0)
    out = np.ascontiguousarray(out.astype(np.float32))
    if _trace:
        return out, res
    return out


# revision 4
# speedup vs baseline: 1.3300x; 1.3300x over previous
"""Trainium2 Bass kernel for nn_Encoder_Flows (3-layer dense GCN message passing).

Math per graph (reference):
    A = flows [N, N];  deg[c] = sum_r A[r, c];  dinv = rsqrt(deg)
    L(x, W, b) = dinv * (A^T @ (dinv * (x @ W))) + b
    out = L(L(L(A, W1, b1), W2, b2), W3, b3)          # [N, 128]

Strategy: data-parallel over the batch (16 graphs / 8 cores = 2 graphs per
core, processed sequentially inside one NEFF). A is cast to bf16 on the host
and kept resident in SBUF (double-buffered across graphs). The layer-1
feature matmul A @ W1 needs A-transposed tiles; those come from hardware
DMA-transpose (bf16-only feature) streamed per 128-column strip. All other
stages pick matmul operand roles so that no on-chip transpose is ever
needed:
  u1   = A @ W1          : lhsT = At-strip tiles (stationary), rhs = W1  -> node-major
  y1   = A^T @ msg1      : lhsT = msg1, rhs = A (N=512 chunks)           -> feat-major
  v2   = y1 @ W2         : lhsT = y1T tiles (feat-major is the lhsT!)    -> node-major
  y2   = A^T @ msg2      : like y1, two 128-col halves                   -> feat-major
  v3   = y2 @ W3         : lhsT = y2T tiles                              -> node-major
  y3   = A^T @ msg3      : lhsT = A tiles (stationary), rhs = msg3       -> node-major
deg comes free as a DVE free-axis reduction over the DMA-transposed strips.
All matmuls accumulate in fp32 PSUM; dinv/scalings in fp32.
"""

import sys
from contextlib import ExitStack

import numpy as np

for _p in ("/opt/trn_rl_repo", "/opt/pypackages"):
    if _p not in sys.path:
        sys.path.append(_p)

import ml_dtypes

B, N, P = 16, 2048, 128
NB = N // P          # 16 row/col blocks
NCORES = 8
GPC = B // NCORES    # graphs per core
D1, D2, D3 = 128, 256, 128
CH = 512             # moving-operand chunk
NCH = N // CH

_COMPILED = {}


def _build(with_bias):
    import concourse.mybir as mybir
    import concourse.tile as tile
    from concourse import bacc

    f32 = mybir.dt.float32
    bf16 = mybir.dt.bfloat16

    nc = bacc.Bacc("TRN2", target_bir_lowering=False)
    Ab_d = nc.declare_dram_parameter("Ab", [GPC, N, N], bf16, isOutput=False)
    W1_d = nc.declare_dram_parameter("W1b", [N, D1], bf16, isOutput=False)
    W2_d = nc.declare_dram_parameter("W2b", [D1, D2], bf16, isOutput=False)
    W3_d = nc.declare_dram_parameter("W3b", [D2, D3], bf16, isOutput=False)
    if with_bias:
        c2_d = nc.declare_dram_parameter("c2r", [P, D2], f32, isOutput=False)
        c3_d = nc.declare_dram_parameter("c3r", [P, D3], f32, isOutput=False)
        b3_d = nc.declare_dram_parameter("b3r", [P, D3], f32, isOutput=False)
    out_d = nc.declare_dram_parameter("out", [GPC, N, D3], f32, isOutput=True)

    with tile.TileContext(nc) as tc, ExitStack() as ctx:
        X = mybir.AxisListType.X
        wpool = ctx.enter_context(tc.tile_pool(name="wpool", bufs=1))
        spool = ctx.enter_context(tc.tile_pool(name="spool", bufs=2))
        apool = ctx.enter_context(tc.tile_pool(name="apool", bufs=2))
        strips = ctx.enter_context(tc.tile_pool(name="strips", bufs=6))
        mpool = ctx.enter_context(tc.tile_pool(name="mpool", bufs=1))
        y2pool = ctx.enter_context(tc.tile_pool(name="y2pool", bufs=1))
        psum = ctx.enter_context(tc.tile_pool(name="psum", bufs=1, space="PSUM"))
        psh = ctx.enter_context(tc.tile_pool(name="psh", bufs=4, space="PSUM"))

        # --- weights, replicated constants ---
        W1_sb = wpool.tile([P, NB, D1], bf16)
        nc.sync.dma_start(W1_sb[:], W1_d.ap().rearrange("(fb p) d -> p fb d", p=P))
        W2_sb = wpool.tile([P, D2], bf16)
        nc.sync.dma_start(W2_sb[:], W2_d.ap())
        W3_sb = wpool.tile([P, 2, D3], bf16)
        nc.sync.dma_start(W3_sb[:], W3_d.ap().rearrange("(h p) g -> p h g", p=P))
        if with_bias:
            c2_sb = wpool.tile([P, D2], f32)
            nc.sync.dma_start(c2_sb[:], c2_d.ap())
            c3_sb = wpool.tile([P, D3], f32)
            nc.sync.dma_start(c3_sb[:], c3_d.ap())
            b3_sb = wpool.tile([P, D3], f32)
            nc.sync.dma_start(b3_sb[:], b3_d.ap())

        iob = wpool.tile([P, P], bf16)
        from concourse.masks import make_identity
        make_identity(nc, iob[:])

        out_ap = out_d.ap().rearrange("g (cb p) d -> g p cb d", p=P)

        for g in range(GPC):
            # ---------- load A (bf16, natural layout) ----------
            A_sb = apool.tile([P, NB, N], bf16, tag="A")
            nc.sync.dma_start(A_sb[:], Ab_d.ap()[g].rearrange("(rb p) c -> p rb c", p=P))

            deg = spool.tile([P, NB], f32, tag="deg")
            dinv = spool.tile([P, NB], f32, tag="dinv")
            rdeg = spool.tile([P, NB], f32, tag="rdeg")

            # ---------- u1T = (A @ W1)^T via DMA-transposed strips; deg free --
            # u1T[d, m] accumulates over fb: lhsT = W1[fb], rhs = At-strip chunks
            u1t = psum.tile([P, N], f32, tag="big")
            for fb in range(NB):
                strip = strips.tile([P, N], bf16, tag="strip")
                nc.sync.dma_start_transpose(strip[:], Ab_d.ap()[g][:, fb * P:(fb + 1) * P])
                nc.vector.reduce_sum(deg[:, fb:fb + 1], strip[:], axis=X)
                for ch in range(NCH):
                    nc.tensor.matmul(
                        u1t[:, ch * CH:(ch + 1) * CH], W1_sb[:, fb, :],
                        strip[:, ch * CH:(ch + 1) * CH],
                        start=(fb == 0), stop=(fb == NB - 1))

            # dinv = sqrt(1/deg); rdeg = 1/deg = dinv^2
            nc.vector.reciprocal(rdeg[:], deg[:])
            nc.scalar.sqrt(dinv[:], rdeg[:])

            # ---------- msg1 = dinv * u1 (node-major via 16 PE transposes) ----
            msg1 = mpool.tile([P, NB, D1], bf16, tag="msg")
            for q in range(4):
                u1q = spool.tile([P, CH], bf16, tag="u1q")
                nc.vector.tensor_copy(u1q[:], u1t[:, q * CH:(q + 1) * CH])
                pt = psh.tile([P, 4, P], bf16, tag="sh")
                for j in range(4):
                    nc.tensor.transpose(pt[:, j, :], u1q[:, j * P:(j + 1) * P], iob[:])
                sl = slice(q * 4, (q + 1) * 4)
                nc.vector.tensor_tensor(
                    msg1[:, sl, :], pt[:],
                    dinv[:, sl, None].to_broadcast([P, 4, D1]),
                    mybir.AluOpType.mult)

            # ---------- y1 = A^T @ msg1 (chunked); v2 = y1 @ W2; msg2 -------
            msg2 = mpool.tile([P, NB, D2], bf16, tag="msg2")
            for ch in range(NCH):
                y1c = psh.tile([P, CH], f32, tag="sh")
                for rb in range(NB):
                    nc.tensor.matmul(
                        y1c[:], msg1[:, rb, :],
                        A_sb[:, rb, ch * CH:(ch + 1) * CH],
                        start=(rb == 0), stop=(rb == NB - 1))
                y1q = spool.tile([P, CH], bf16, tag="y1q")
                nc.vector.tensor_copy(y1q[:], y1c[:])
                for j in range(4):
                    nb = ch * 4 + j
                    v2p = psh.tile([P, D2], f32, tag="sh")
                    nc.tensor.matmul(v2p[:], y1q[:, j * P:(j + 1) * P], W2_sb[:],
                                     start=True, stop=True)
                    if with_bias:
                        t = spool.tile([P, D2], f32, tag="tbias")
                        nc.vector.tensor_tensor(
                            t[:], v2p[:], dinv[:, nb:nb + 1].to_broadcast([P, D2]),
                            mybir.AluOpType.mult)
                        nc.vector.tensor_tensor(t[:], t[:], c2_sb[:], mybir.AluOpType.add)
                        nc.vector.tensor_tensor(
                            msg2[:, nb, :], t[:], dinv[:, nb:nb + 1].to_broadcast([P, D2]),
                            mybir.AluOpType.mult)
                    else:
                        nc.vector.tensor_tensor(
                            msg2[:, nb, :], v2p[:], rdeg[:, nb:nb + 1].to_broadcast([P, D2]),
                            mybir.AluOpType.mult)

            # ---------- y2 = A^T @ msg2 (two halves, chunked psum) ----------
            y2h = []
            for half in range(2):
                yh = y2pool.tile([P, N], bf16, tag=f"y2h{half}")
                for ch in range(NCH):
                    y2c = psh.tile([P, CH], f32, tag="sh")
                    for rb in range(NB):
                        nc.tensor.matmul(
                            y2c[:],
                            msg2[:, rb, half * P:(half + 1) * P],
                            A_sb[:, rb, ch * CH:(ch + 1) * CH],
                            start=(rb == 0), stop=(rb == NB - 1))
                    nc.vector.tensor_copy(yh[:, ch * CH:(ch + 1) * CH], y2c[:])
                y2h.append(yh)

            # ---------- v3 = y2 @ W3 ; msg3 = rdeg*v3 (+ dinv*c3) ----------
            msg3 = mpool.tile([P, NB, D3], bf16, tag="msg")
            for nb in range(NB):
                v3p = psh.tile([P, D3], f32, tag="sh")
                for half in range(2):
                    nc.tensor.matmul(v3p[:], y2h[half][:, nb * P:(nb + 1) * P],
                                     W3_sb[:, half, :],
                                     start=(half == 0), stop=(half == 1))
                if with_bias:
                    t3 = spool.tile([P, D3], f32, tag="tbias3")
                    nc.vector.tensor_tensor(
                        t3[:], v3p[:], dinv[:, nb:nb + 1].to_broadcast([P, D3]),
                        mybir.AluOpType.mult)
                    nc.vector.tensor_tensor(t3[:], t3[:], c3_sb[:], mybir.AluOpType.add)
                    nc.vector.tensor_tensor(
                        msg3[:, nb, :], t3[:], dinv[:, nb:nb + 1].to_broadcast([P, D3]),
                        mybir.AluOpType.mult)
                else:
                    nc.vector.tensor_tensor(
                        msg3[:, nb, :], v3p[:], rdeg[:, nb:nb + 1].to_broadcast([P, D3]),
                        mybir.AluOpType.mult)

            # ---------- y3 = A^T @ msg3 (A-stationary, grouped) + out -------
            for qg in range(4):
                y3g = psh.tile([P, 4, P], f32, tag="sh")
                for j in range(4):
                    cb = qg * 4 + j
                    for rb in range(NB):
                        nc.tensor.matmul(
                            y3g[:, j, :], A_sb[:, rb, cb * P:(cb + 1) * P], msg3[:, rb, :],
                            start=(rb == 0), stop=(rb == NB - 1))
                sl = slice(qg * 4, (qg + 1) * 4)
                og = spool.tile([P, 4, D3], f32, tag="og")
                nc.vector.tensor_tensor(
                    og[:], y3g[:],
                    dinv[:, sl, None].to_broadcast([P, 4, D3]),
                    mybir.AluOpType.mult)
                if with_bias:
                    nc.vector.tensor_tensor(
                        og[:], og[:], b3_sb[:, None, :].to_broadcast([P, 4, D3]),
                        mybir.AluOpType.add)
                nc.sync.dma_start(out_ap[g][:, sl, :], og[:])

    nc.compile()
    return nc


def _get_nc(with_bias):
    key = bool(with_bias)
    if key not in _COMPILED:
        _COMPILED[key] = _build(key)
    return _COMPILED[key]


def kernel(flows, W1, b1, W2, b2, W3, b3, _trace=False):
    from concourse.bass_utils import run_bass_kernel_spmd

    flows = np.asarray(flows, dtype=np.float32)
    W1 = np.asarray(W1, dtype=np.float32)
    W2 = np.asarray(W2, dtype=np.float32)
    W3 = np.asarray(W3, dtype=np.float32)
    b1 = np.asarray(b1, dtype=np.float32)
    b2 = np.asarray(b2, dtype=np.float32)
    b3 = np.asarray(b3, dtype=np.float32)

    with_bias = bool(np.any(b1) or np.any(b2) or np.any(b3))
    nc = _get_nc(with_bias)

    Ab = flows.astype(ml_dtypes.bfloat16)
    W1b = W1.astype(ml_dtypes.bfloat16)
    W2b = W2.astype(ml_dtypes.bfloat16)
    W3b = W3.astype(ml_dtypes.bfloat16)

    in_maps = []
    for c in range(NCORES):
        m = {
            "Ab": Ab[c * GPC:(c + 1) * GPC],
            "W1b": W1b, "W2b": W2b, "W3b": W3b,
        }
        if with_bias:
            m["c2r"] = np.broadcast_to(b1 @ W2, (P, D2)).copy().astype(np.float32)
            m["c3r"] = np.broadcast_to(b2 @ W3, (P, D3)).copy().astype(np.float32)
            m["b3r"] = np.broadcast_to(b3, (P, D3)).copy().astype(np.float32)
        in_maps.append(m)

    res = run_bass_kernel_spmd(nc, in_maps, core_ids=list(range(NCORES)), trace=_trace)
    out = np.concatenate([res.results[c]["out"] for c in range(NCORES)], axis=0)
    out = np.ascontiguousarray(out.astype(np.float32))
    if _trace:
        return out, res
    return out
